# revision 41
# baseline (speedup 1.0000x reference)
"""Causal self-attention (B=2, N=2048, D=2048, H=16, hd=128) on 8 Trainium2
NeuronCores.

Strategy (tensor-parallel over heads, 2 heads/core), v2:
  - Host: transpose x / weights, build RoPE tables + triangular mask consts,
    slice w_qkv rows per head-group.
  - Device, per core (same SPMD program, different input data):
    Phase A: qkvT projection (bf16 matmuls, outputs in [d, n] layout) + RoPE
             (DVE mul/add on psum pairs) -> stage tiles -> SBUF->SBUF DMA
             repack into per-head [128=hd, N] q/k tiles (full-contract
             scores).
    Phase B: S.T = kh.T @ qh in ONE c=128 matmul per tile, P.T = exp(S.T)
             (ACT), causal mask via sliced triangular const (DVE), O.T
             accumulated as vT.T @ P.T (PE, PSUM accum).  Softmax denoms:
             ones-column matmul accumulated in PSUM over t (PE), fast
             reciprocal (DVE custom op), partition_broadcast (GPSIMD),
             final scale (DVE).
    Per-batch AllToAll (256-row chunks to each core) fired right after each
    batch's attention: b0's collective hides under b1's compute.
    Phase C: o_proj on the 2x256-row shard with w_o pre-cached in SBUF
             (8MB DMA issued during b0's attention); first half overlaps
             b1's collective.
  - Host: reassemble [b0 rows 256c:256c+256 | b1 rows 256c:256c+256].
"""

import sys
import time

import ml_dtypes
import numpy as np

sys.path.insert(0, "/opt/trn_rl_repo")

import concourse.bacc as bacc  # noqa: E402
import concourse.bass as bass  # noqa: E402
import concourse.mybir as mybir  # noqa: E402
import concourse.tile as tile  # noqa: E402
from concourse import bass_utils  # noqa: E402

F32 = mybir.dt.float32
BF16 = mybir.dt.bfloat16

B, N, D = 2, 2048, 2048
H, HD = 16, 128
NC = 8
HPC = H // NC          # heads per core
BN = B * N             # 4096
NSH = BN // NC         # output rows per core
INNER = H * HD
ROPE_BASE = 10000.0

_CACHE = {}

LAST_EXEC_NS = None
LAST_RESULTS = None


def _build_program():
    nc = bacc.Bacc(
        "TRN2",
        target_bir_lowering=False,
        debug=False,
        enable_asserts=False,
        num_devices=NC,
    )
    xT = nc.dram_tensor("xT", [D, BN], BF16, kind="ExternalInput").ap()
    wqkT = nc.dram_tensor("wqkT", [D, 4 * HD], BF16, kind="ExternalInput").ap()
    wvT = nc.dram_tensor("wvT", [D, HPC * HD], BF16, kind="ExternalInput").ap()
    woT = nc.dram_tensor("woT", [INNER, D], BF16, kind="ExternalInput").ap()
    tabs = nc.dram_tensor("tabs", [4, HD, BN], BF16, kind="ExternalInput").ap()
    tri = nc.dram_tensor("tri", [128, 1024], BF16, kind="ExternalInput").ap()
    ident = nc.dram_tensor("ident", [128, 128], BF16, kind="ExternalInput").ap()
    # contiguous copy of the first x chunk: the startup-critical DMA runs at
    # full rate instead of the strided gather pattern.
    xf0 = nc.dram_tensor("xf0", [2, 128, 8, 512], BF16, kind="ExternalInput").ap()
    out = nc.dram_tensor("out", [NSH, D], F32, kind="ExternalOutput").ap()
    a2a_in = [
        [
            nc.dram_tensor(f"a2a_in{b}_{h}", [NC, 128, 256], BF16).ap()
            for h in range(HPC)
        ]
        for b in range(B)
    ]
    a2a_out = [
        [
            nc.dram_tensor(f"a2a_out{b}_{h}", [NC, 128, 256], BF16).ap()
            for h in range(HPC)
        ]
        for b in range(B)
    ]

    MUL = mybir.AluOpType.mult
    ADD = mybir.AluOpType.add
    SUB = mybir.AluOpType.subtract
    EXP = mybir.ActivationFunctionType.Exp

    with tile.TileContext(nc, num_cores=NC) as tc:
        with (
            tc.tile_pool(name="const", bufs=1) as constp,
            tc.tile_pool(name="wqk", bufs=1) as wqkp,
            tc.tile_pool(name="wv", bufs=1) as wvp,
            tc.tile_pool(name="wo", bufs=1) as wop,
            tc.tile_pool(name="persist", bufs=1) as persist,
        ):
            wqk_sb = wqkp.tile([128, 16, 512], BF16, name="wqk_sb")
            wv_sb = wvp.tile([128, 16, 256], BF16, name="wv_sb")
            wo_sb = wop.tile([128, 16, D], BF16, name="wo_sb")
            tri_sb = constp.tile([128, 1024], BF16, name="tri_sb")
            ident_sb = constp.tile([128, 128], BF16, name="ident_sb")
            ones_col = constp.tile([128, 1], BF16, name="ones_col")

            with (
                tc.tile_pool(name="xt", bufs=5) as xtp,
                tc.tile_pool(name="tab", bufs=2) as tabp,
                tc.tile_pool(name="rope", bufs=2) as ropep,
                tc.tile_pool(name="stage", bufs=3) as stagep,
                tc.tile_pool(name="pt", bufs=4) as ptp,
                tc.tile_pool(name="small", bufs=2) as smallp,
                tc.tile_pool(name="ots", bufs=2) as otsp,
                tc.tile_pool(name="rsc", bufs=2) as rscp,
                tc.tile_pool(name="pst", bufs=2, space="PSUM") as pstp,
                tc.tile_pool(name="pov", bufs=2, space="PSUM") as povp,
                tc.tile_pool(name="psmall", bufs=2, space="PSUM") as psmallp,
            ):
                def load_x(b, j):
                    n0 = b * N + 512 * j
                    xh = []
                    for half in range(2):
                        t = xtp.tile(
                            [128, 8, 512], BF16, tag="xt", name=f"xt_{b}_{j}_{half}"
                        )
                        nc.sync.dma_start(
                            out=t[:, :, :],
                            in_=xT.rearrange("(k p) n -> p k n", p=128)[
                                :, 8 * half : 8 * half + 8, n0 : n0 + 512
                            ],
                        )
                        xh.append(t)
                    return xh

                # first x chunk before the weights: the first matmul chain
                # is gated on xh(0,0) + the low wqk half only.
                xh_first = []
                for half in range(2):
                    t = xtp.tile([128, 8, 512], BF16, tag="xt", name=f"xt_0_0_{half}")
                    nc.sync.dma_start(out=t[:, :, :], in_=xf0[half])
                    xh_first.append(t)
                nc.sync.dma_start(
                    out=wqk_sb[:, 0:8, :],
                    in_=wqkT.rearrange("(k p) m -> p k m", p=128)[:, 0:8, :],
                )
                nc.sync.dma_start(
                    out=wqk_sb[:, 8:16, :],
                    in_=wqkT.rearrange("(k p) m -> p k m", p=128)[:, 8:16, :],
                )
                nc.sync.dma_start(
                    out=wv_sb[:, :, :],
                    in_=wvT.rearrange("(k p) m -> p k m", p=128),
                )
                nc.sync.dma_start(out=tri_sb[:, :], in_=tri[:, :])
                nc.sync.dma_start(out=ident_sb[:, :], in_=ident[:, :])
                nc.vector.memset(ones_col[:, :], 1.0)

                def qk_pair(b, j, pair, xh, qh_sb, tabt):
                    pw = pstp.tile([128, 1024], F32, tag="pst", name=f"pw_{b}_{j}_{pair}")
                    psA = pw[:, 0:512]
                    psB = pw[:, 512:1024]
                    for mt, pst_ in ((pair, psA), (pair + 1, psB)):
                        for k in range(16):
                            nc.tensor.matmul(
                                pst_,
                                lhsT=(wqk_sb[:, k, 128 * mt : 128 * mt + 128]),
                                rhs=(xh[k // 8][:, k % 8, :]),
                                start=(k == 0),
                                stop=(k == 15),
                            )
                    ci = 0 if pair == 0 else 2
                    t1 = ropep.tile([128, 512], BF16, tag="t1", name=f"t1_{b}_{j}_{pair}")
                    t2 = ropep.tile([128, 512], BF16, tag="t2", name=f"t2_{b}_{j}_{pair}")
                    t3 = ropep.tile([128, 512], BF16, tag="t3", name=f"t3_{b}_{j}_{pair}")
                    t4 = ropep.tile([128, 512], BF16, tag="t4", name=f"t4_{b}_{j}_{pair}")
                    nc.vector.tensor_tensor(t1[:, :], psA, tabt[ci][:, :], MUL)
                    nc.vector.tensor_tensor(t2[:, :], psB, tabt[ci + 1][:, :], MUL)
                    nc.vector.tensor_tensor(t3[:, :], psB, tabt[ci][:, :], MUL)
                    nc.vector.tensor_tensor(t4[:, :], psA, tabt[ci + 1][:, :], MUL)
                    sl = stagep.tile([128, 512], BF16, tag="sl", name=f"sl_{b}_{j}_{pair}")
                    sh = stagep.tile([128, 512], BF16, tag="sh", name=f"sh_{b}_{j}_{pair}")
                    nc.vector.tensor_tensor(sl[:, :], t1[:, :], t2[:, :], SUB)
                    nc.vector.tensor_tensor(sh[:, :], t3[:, :], t4[:, :], ADD)
                    # repack: per-head [lo;hi] tiles for full-contract
                    # scores.  base tile index: q -> 0, k -> 2.
                    base = 0 if pair == 0 else 2
                    cs = slice(512 * j, 512 * (j + 1))
                    nc.sync.dma_start(out=qh_sb[0:64, base, cs], in_=sl[0:64, :])
                    nc.sync.dma_start(out=qh_sb[0:64, base + 1, cs], in_=sl[64:128, :])
                    nc.sync.dma_start(out=qh_sb[64:128, base, cs], in_=sh[0:64, :])
                    nc.sync.dma_start(out=qh_sb[64:128, base + 1, cs], in_=sh[64:128, :])

                def phase_a_chunk(b, j, qh_sb, vT_sb):
                    n0 = b * N + 512 * j
                    xh = xh_first if (b, j) == (0, 0) else load_x(b, j)
                    tabt = []
                    for ti in range(4):
                        tt = tabp.tile([128, 512], BF16, tag=f"tab{ti}", name=f"tab{ti}_{b}_{j}")
                        nc.sync.dma_start(out=tt[:, :], in_=tabs[ti, :, n0 : n0 + 512])
                        tabt.append(tt)
                    for pair in (0, 2):
                        qk_pair(b, j, pair, xh, qh_sb, tabt)
                    for mt in range(4):
                        pv = povp.tile([128, 256], F32, tag="pov", name=f"psV_{b}_{j}_{mt}")
                        for k in range(16):
                            nc.tensor.matmul(
                                pv[:, :],
                                lhsT=(xh[k // 8][:, k % 8, 128 * mt : 128 * mt + 128]),
                                rhs=(wv_sb[:, k, :]),
                                start=(k == 0),
                                stop=(k == 15),
                            )
                        nc.scalar.copy(vT_sb[:, 4 * j + mt, :], pv[:, :])

                def score_pair(b, h, j, m, qh_sb):
                    pw = pstp.tile(
                        [128, 1024], F32, tag="pst", name=f"stw_{b}_{h}_{j}_{m}"
                    )
                    for half in range(2):
                        t = 2 * m + half
                        sl = pw[:, 512 * half : 512 * half + 512]
                        diag = t // 4 == j
                        if diag:
                            # causal mask as a -30 additive bias matmul
                            # (identity stationary): no DVE pass after exp.
                            f0 = 128 * t - 512 * j
                            nc.tensor.matmul(
                                sl,
                                lhsT=ident_sb[:, :],
                                rhs=tri_sb[:, 512 - f0 : 1024 - f0],
                                start=True,
                                stop=False,
                            )
                        nc.tensor.matmul(
                            sl,
                            lhsT=(qh_sb[:, 2 + h, 128 * t : 128 * t + 128]),
                            rhs=(qh_sb[:, h, 512 * j : 512 * (j + 1)]),
                            start=not diag,
                            stop=True,
                        )
                    ptw = ptp.tile(
                        [128, 1024], BF16, tag="pt", name=f"pt_{b}_{h}_{j}_{m}"
                    )
                    nc.scalar.activation(ptw[:, :], pw[:, :], EXP)
                    return ptw

                def phase_b_unit(b, h, j, qh_sb, vT_sb):
                    ov = povp.tile([128, 512], F32, tag="pov", name=f"ov_{b}_{h}_{j}")
                    # row-sum path: j=0 accumulates per-tile on PE (PSUM),
                    # j>=1 accumulates elementwise on DVE -- balances PE
                    # against ACT/DVE in this phase.
                    on_pe = j == 0
                    rsum = None
                    rs_c = None
                    if on_pe:
                        rsum = psmallp.tile(
                            [1, 512], F32, tag="rsum", name=f"rsum_{b}_{h}_{j}"
                        )
                    else:
                        rs_c = rscp.tile(
                            [128, 512], F32, tag="rsc", name=f"rsc_{b}_{h}_{j}"
                        )
                    nt = 4 * j + 4
                    for m in range(nt // 2):
                        ptw = score_pair(b, h, j, m, qh_sb)
                        if on_pe:
                            for half in range(2):
                                t = 2 * m + half
                                nc.tensor.matmul(
                                    rsum[:, :],
                                    lhsT=ones_col[:, :],
                                    rhs=ptw[:, 512 * half : 512 * half + 512],
                                    start=(t == 0),
                                    stop=(t == nt - 1),
                                )
                        elif m == 0:
                            nc.vector.tensor_tensor(
                                rs_c[:, :], ptw[:, 0:512], ptw[:, 512:1024], ADD
                            )
                        else:
                            nc.vector.tensor_tensor(
                                rs_c[:, :], rs_c[:, :], ptw[:, 0:512], ADD
                            )
                            nc.vector.tensor_tensor(
                                rs_c[:, :], rs_c[:, :], ptw[:, 512:1024], ADD
                            )
                        for half in range(2):
                            t = 2 * m + half
                            nc.tensor.matmul(
                                ov[:, :],
                                lhsT=(vT_sb[:, t, 128 * h : 128 * h + 128]),
                                rhs=(ptw[:, 512 * half : 512 * half + 512]),
                                start=(t == 0),
                                stop=(t == nt - 1),
                            )
                    if not on_pe:
                        rs_bf = rscp.tile(
                            [128, 512], BF16, tag="rsbf", name=f"rsbf_{b}_{h}_{j}"
                        )
                        nc.scalar.copy(rs_bf[:, :], rs_c[:, :])
                        rsum = psmallp.tile(
                            [1, 512], F32, tag="rsum", name=f"rsum_{b}_{h}_{j}"
                        )
                        nc.tensor.matmul(
                            rsum[:, :],
                            lhsT=ones_col[:, :],
                            rhs=rs_bf[:, :],
                            start=True,
                            stop=True,
                        )
                    rinv = smallp.tile([1, 512], F32, tag="rinv", name=f"rinv_{b}_{h}_{j}")
                    nc.vector.reciprocal_approx_fast(rinv[:, :], rsum[:, :])
                    binv = smallp.tile(
                        [128, 512], F32, tag="binv", name=f"binv_{b}_{h}_{j}"
                    )
                    nc.gpsimd.partition_broadcast(binv[:, :], rinv[:, :])
                    ot = otsp.tile([128, 512], BF16, tag="ot", name=f"ot_{b}_{h}_{j}")
                    nc.vector.tensor_tensor(ot[:, :], ov[:, :], binv[:, :], MUL)
                    nc.sync.dma_start(
                        out=a2a_in[b][h][2 * j, :, :], in_=ot[:, 0:256]
                    )
                    nc.sync.dma_start(
                        out=a2a_in[b][h][2 * j + 1, :, :], in_=ot[:, 256:512]
                    )

                def emit_cc(b, h):
                    # per-(batch,head) reshard: all but the last collective
                    # hide under subsequent compute; the last hides under
                    # phase C's first chains.
                    nc.gpsimd.collective_compute(
                        "AllToAll",
                        mybir.AluOpType.bypass,
                        replica_groups=[list(range(NC))],
                        ins=[a2a_in[b][h].opt()],
                        outs=[a2a_out[b][h].opt()],
                    )

                # collectives are emitted one compute-unit late so the
                # gpsimd trigger doesn't block the next unit's
                # partition_broadcast.
                pending_cc = None
                for b in range(B):
                    qh_sb = persist.tile(
                        [128, 4, N], BF16, tag="qh", name=f"qh_b{b}"
                    )
                    vT_sb = persist.tile(
                        [128, 16, HPC * HD], BF16, tag="vT", name=f"vT_b{b}"
                    )
                    # ---------------- phase A: projection + RoPE ----------
                    for j in range(4):
                        phase_a_chunk(b, j, qh_sb, vT_sb)
                        if j == 0 and pending_cc is not None:
                            emit_cc(*pending_cc)
                            pending_cc = None

                    if b == 0:
                        # w_o cache fill: issued here so the 8MB transfer
                        # rides under b0's attention, clear of x loads.
                        nc.sync.dma_start(
                            out=wo_sb[:, :, :],
                            in_=woT.rearrange("(k p) d -> p k d", p=128),
                        )

                    # ---------------- phase B: attention ------------------
                    for h in range(HPC):
                        for j in range(4):
                            phase_b_unit(b, h, j, qh_sb, vT_sb)
                            if j == 0 and pending_cc is not None:
                                emit_cc(*pending_cc)
                                pending_cc = None
                        pending_cc = (b, h)
                if pending_cc is not None:
                    emit_cc(*pending_cc)
                    pending_cc = None

            # ---------------- phase C: o_proj ------------------------------
            # opin k-tile order is (h, src) -> woT rows are host-permuted to
            # match.  Output rows are staged in full-width tiles so each
            # 128-row block ships as one large DMA.
            with (
                tc.tile_pool(name="opin", bufs=1) as opinp,
                tc.tile_pool(name="outs", bufs=4) as outsp,
                tc.tile_pool(name="pc", bufs=8, space="PSUM") as pcp,
            ):
                opins = []
                for half in range(B):
                    opin = opinp.tile([128, 16, 256], BF16, tag=f"opin{half}", name=f"opin{half}")
                    for h in range(HPC):
                        nc.sync.dma_start(
                            out=opin[:, 8 * h : 8 * h + 8, :],
                            in_=a2a_out[half][h].rearrange("r p n -> p r n"),
                        )
                    opins.append(opin)
                def c_chain(half, ns, dc, pc, k0, k1):
                    opin = opins[half]
                    for k in range(k0, k1):
                        nc.tensor.matmul(
                            pc[:, :],
                            lhsT=(opin[:, k, 128 * ns : 128 * ns + 128]),
                            rhs=(wo_sb[:, k, 512 * dc : 512 * (dc + 1)]),
                            start=(k == 0),
                            stop=(k == 15),
                        )

                def c_finish(half, ns, dc, pc, ost):
                    nc.scalar.copy(ost[:, 512 * dc : 512 * (dc + 1)], pc[:, :])
                    r0 = 256 * half + 128 * ns
                    nc.sync.dma_start(
                        out=out[r0 : r0 + 128, 512 * dc : 512 * (dc + 1)],
                        in_=ost[:, 512 * dc : 512 * (dc + 1)],
                    )

                osts = {}
                for ns in range(2):
                    osts[0, ns] = outsp.tile([128, D], F32, tag="outs", name=f"os_0_{ns}")
                    for dc in range(4):
                        pc = pcp.tile([128, 512], F32, tag="pc", name=f"pc_0_{dc}_{ns}")
                        c_chain(0, ns, dc, pc, 0, 16)
                        c_finish(0, ns, dc, pc, osts[0, ns])
                # half 1: all chains split at the k=8 boundary -- the low
                # halves depend only on the earlier (b1,h0) collective, so
                # they fill the window while the last collective completes.
                pcs = {}
                for ns in range(2):
                    osts[1, ns] = outsp.tile([128, D], F32, tag="outs", name=f"os_1_{ns}")
                    for dc in range(4):
                        pcs[ns, dc] = pcp.tile([128, 512], F32, tag="pc", name=f"pc_1_{dc}_{ns}")
                        c_chain(1, ns, dc, pcs[ns, dc], 0, 8)
                for ns in range(2):
                    for dc in range(4):
                        c_chain(1, ns, dc, pcs[ns, dc], 8, 16)
                        c_finish(1, ns, dc, pcs[ns, dc], osts[1, ns])
    nc.compile()
    return nc


def _host_prep(x, w_qkv, w_o):
    bf = ml_dtypes.bfloat16
    xT = np.ascontiguousarray(x.reshape(BN, D).T).astype(bf)
    # o_proj k-tile order on device is (h_local, src_core): head g lives at
    # slot 8*(g % 2) + g // 2.
    woT_n = np.asarray(w_o).T.reshape(H, HD, D)
    perm = [2 * s + hl for hl in range(HPC) for s in range(NC)]
    woT = np.ascontiguousarray(woT_n[perm].reshape(INNER, D)).astype(bf)

    inv_freq = 1.0 / (ROPE_BASE ** (np.arange(0, HD, 2, dtype=np.float32) / HD))
    ang = np.arange(N, dtype=np.float32)[:, None] * inv_freq[None, :]
    cos_h = np.cos(ang).T.astype(np.float32)      # [64, N]
    sin_h = np.sin(ang).T.astype(np.float32)      # [64, N] (magnitude)
    # duplicated for the two heads packed per 128-row block
    cos2 = np.concatenate([cos_h, cos_h], axis=0)  # [128, N]
    sin2 = np.concatenate([sin_h, sin_h], axis=0)
    cos_f = np.tile(cos2, (1, B))
    sin_f = np.tile(sin2, (1, B))
    scale = np.float32(1.0 / np.sqrt(HD))
    tabs = np.ascontiguousarray(
        np.stack([cos_f * scale, sin_f * scale, cos_f, sin_f], axis=0)
    ).astype(bf)

    xf0 = np.ascontiguousarray(
        xT.reshape(16, 128, BN)[:, :, 0:512]
        .reshape(2, 8, 128, 512)
        .transpose(0, 2, 1, 3)
    )

    p = np.arange(128)[:, None]
    c = np.arange(1024)[None, :]
    # additive causal bias: 0 where visible, -30 where masked (exp -> ~0)
    tri = np.where(p <= c - 512, 0.0, -30.0).astype(bf)
    ident = np.eye(128, dtype=np.float32).astype(bf)

    in_maps = []
    for core in range(NC):
        h0 = core * HPC
        rq = slice(h0 * HD, (h0 + HPC) * HD)
        rk = slice(INNER + h0 * HD, INNER + (h0 + HPC) * HD)
        rv = slice(2 * INNER + h0 * HD, 2 * INNER + (h0 + HPC) * HD)
        wq = w_qkv[rq].reshape(HPC, HD, D)
        wk = w_qkv[rk].reshape(HPC, HD, D)
        # row order per block: [h0_lo, h1_lo | h0_hi, h1_hi] for q then k
        wqkT = np.ascontiguousarray(
            np.concatenate(
                [wq[0, :64], wq[1, :64], wq[0, 64:], wq[1, 64:],
                 wk[0, :64], wk[1, :64], wk[0, 64:], wk[1, 64:]], axis=0
            ).T
        ).astype(bf)
        wvT = np.ascontiguousarray(w_qkv[rv].T).astype(bf)
        in_maps.append(
            dict(xT=xT, wqkT=wqkT, wvT=wvT, woT=woT, tabs=tabs, tri=tri,
                 ident=ident, xf0=xf0)
        )
    return in_maps


def kernel(x, w_qkv, w_o, n_heads=None, head_dim=None, trace=False):
    global LAST_EXEC_NS, LAST_RESULTS
    x = np.asarray(x, dtype=np.float32)
    w_qkv = np.asarray(w_qkv, dtype=np.float32)
    w_o = np.asarray(w_o, dtype=np.float32)

    if "nc" not in _CACHE:
        _CACHE["nc"] = _build_program()
    nc = _CACHE["nc"]

    in_maps = _host_prep(x, w_qkv, w_o)
    res = None
    last_exc = None
    for attempt in range(4):
        try:
            res = bass_utils.run_bass_kernel_spmd(
                nc, in_maps, core_ids=list(range(NC)), trace=trace
            )
            break
        except Exception as e:  # transient compile_and_load / exec flakiness
            last_exc = e
            print(f"kernel attempt {attempt} failed: {e}", file=sys.stderr)
            time.sleep(5)
    if res is None:
        raise last_exc
    LAST_EXEC_NS = res.exec_time_ns
    LAST_RESULTS = res
    # core c returns [512, D]: rows 0:256 = batch0 rows 256c:256c+256,
    # rows 256:512 = batch1 rows 256c:256c+256.
    full = np.empty((B, N, D), dtype=np.float32)
    for c in range(NC):
        shard = res.results[c]["out"]
        full[0, 256 * c : 256 * c + 256] = shard[0:256]
        full[1, 256 * c : 256 * c + 256] = shard[256:512]
    return full


# revision 43
# speedup vs baseline: 1.0166x; 1.0166x over previous
"""Causal self-attention (B=2, N=2048, D=2048, H=16, hd=128) on 8 Trainium2
NeuronCores.

Strategy (tensor-parallel over heads, 2 heads/core), v2:
  - Host: transpose x / weights, build RoPE tables + triangular mask consts,
    slice w_qkv rows per head-group.
  - Device, per core (same SPMD program, different input data):
    Phase A: qkvT projection (bf16 matmuls, outputs in [d, n] layout) + RoPE
             (DVE mul/add on psum pairs) -> stage tiles -> SBUF->SBUF DMA
             repack into per-head [128=hd, N] q/k tiles (full-contract
             scores).
    Phase B: S.T = kh.T @ qh in ONE c=128 matmul per tile, P.T = exp(S.T)
             (ACT), causal mask via sliced triangular const (DVE), O.T
             accumulated as vT.T @ P.T (PE, PSUM accum).  Softmax denoms:
             ones-column matmul accumulated in PSUM over t (PE), fast
             reciprocal (DVE custom op), partition_broadcast (GPSIMD),
             final scale (DVE).
    Per-batch AllToAll (256-row chunks to each core) fired right after each
    batch's attention: b0's collective hides under b1's compute.
    Phase C: o_proj on the 2x256-row shard with w_o pre-cached in SBUF
             (8MB DMA issued during b0's attention); first half overlaps
             b1's collective.
  - Host: reassemble [b0 rows 256c:256c+256 | b1 rows 256c:256c+256].
"""

import sys
import time

import ml_dtypes
import numpy as np

sys.path.insert(0, "/opt/trn_rl_repo")

import concourse.bacc as bacc  # noqa: E402
import concourse.bass as bass  # noqa: E402
import concourse.mybir as mybir  # noqa: E402
import concourse.tile as tile  # noqa: E402
from concourse import bass_utils  # noqa: E402

F32 = mybir.dt.float32
BF16 = mybir.dt.bfloat16

B, N, D = 2, 2048, 2048
H, HD = 16, 128
NC = 8
HPC = H // NC          # heads per core
BN = B * N             # 4096
NSH = BN // NC         # output rows per core
INNER = H * HD
ROPE_BASE = 10000.0

_CACHE = {}

LAST_EXEC_NS = None
LAST_RESULTS = None


def _build_program():
    nc = bacc.Bacc(
        "TRN2",
        target_bir_lowering=False,
        debug=False,
        enable_asserts=False,
        num_devices=NC,
    )
    xT = nc.dram_tensor("xT", [D, BN], BF16, kind="ExternalInput").ap()
    wqkT = nc.dram_tensor("wqkT", [D, 4 * HD], BF16, kind="ExternalInput").ap()
    wvT = nc.dram_tensor("wvT", [D, HPC * HD], BF16, kind="ExternalInput").ap()
    woT = nc.dram_tensor("woT", [INNER, D], BF16, kind="ExternalInput").ap()
    tabs = nc.dram_tensor("tabs", [4, HD, BN], BF16, kind="ExternalInput").ap()
    tri = nc.dram_tensor("tri", [128, 1024], BF16, kind="ExternalInput").ap()
    ident = nc.dram_tensor("ident", [128, 128], BF16, kind="ExternalInput").ap()
    # contiguous copy of the first x chunk: the startup-critical DMA runs at
    # full rate instead of the strided gather pattern.
    xf0 = nc.dram_tensor("xf0", [2, 128, 8, 512], BF16, kind="ExternalInput").ap()
    out = nc.dram_tensor("out", [NSH, D], F32, kind="ExternalOutput").ap()
    a2a_in = [
        [
            nc.dram_tensor(f"a2a_in{b}_{h}", [NC, 128, 256], BF16).ap()
            for h in range(HPC)
        ]
        for b in range(B)
    ]
    a2a_out = [
        [
            nc.dram_tensor(f"a2a_out{b}_{h}", [NC, 128, 256], BF16).ap()
            for h in range(HPC)
        ]
        for b in range(B)
    ]

    MUL = mybir.AluOpType.mult
    ADD = mybir.AluOpType.add
    SUB = mybir.AluOpType.subtract
    EXP = mybir.ActivationFunctionType.Exp

    with tile.TileContext(nc, num_cores=NC) as tc:
        with (
            tc.tile_pool(name="const", bufs=1) as constp,
            tc.tile_pool(name="wqk", bufs=1) as wqkp,
            tc.tile_pool(name="wv", bufs=1) as wvp,
            tc.tile_pool(name="wo", bufs=1) as wop,
            tc.tile_pool(name="persist", bufs=1) as persist,
        ):
            wqk_sb = wqkp.tile([128, 16, 512], BF16, name="wqk_sb")
            wv_sb = wvp.tile([128, 16, 256], BF16, name="wv_sb")
            wo_sb = wop.tile([128, 16, D], BF16, name="wo_sb")
            tri_sb = constp.tile([128, 1024], BF16, name="tri_sb")
            ident_sb = constp.tile([128, 128], BF16, name="ident_sb")
            ones_col = constp.tile([128, 1], BF16, name="ones_col")

            with (
                tc.tile_pool(name="xt", bufs=5) as xtp,
                tc.tile_pool(name="tab", bufs=2) as tabp,
                tc.tile_pool(name="rope", bufs=2) as ropep,
                tc.tile_pool(name="stage", bufs=3) as stagep,
                tc.tile_pool(name="pt", bufs=4) as ptp,
                tc.tile_pool(name="small", bufs=2) as smallp,
                tc.tile_pool(name="ots", bufs=2) as otsp,
                tc.tile_pool(name="rsc", bufs=2) as rscp,
                tc.tile_pool(name="pst", bufs=2, space="PSUM") as pstp,
                tc.tile_pool(name="pov", bufs=2, space="PSUM") as povp,
                tc.tile_pool(name="psmall", bufs=2, space="PSUM") as psmallp,
            ):
                def load_x(b, j):
                    n0 = b * N + 512 * j
                    xh = []
                    for half in range(2):
                        t = xtp.tile(
                            [128, 8, 512], BF16, tag="xt", name=f"xt_{b}_{j}_{half}"
                        )
                        nc.sync.dma_start(
                            out=t[:, :, :],
                            in_=xT.rearrange("(k p) n -> p k n", p=128)[
                                :, 8 * half : 8 * half + 8, n0 : n0 + 512
                            ],
                        )
                        xh.append(t)
                    return xh

                # first x chunk before the weights: the first matmul chain
                # is gated on xh(0,0) + the low wqk half only.
                xh_first = []
                for half in range(2):
                    t = xtp.tile([128, 8, 512], BF16, tag="xt", name=f"xt_0_0_{half}")
                    nc.sync.dma_start(out=t[:, :, :], in_=xf0[half])
                    xh_first.append(t)
                nc.sync.dma_start(
                    out=wqk_sb[:, 0:8, :],
                    in_=wqkT.rearrange("(k p) m -> p k m", p=128)[:, 0:8, :],
                )
                nc.sync.dma_start(
                    out=wqk_sb[:, 8:16, :],
                    in_=wqkT.rearrange("(k p) m -> p k m", p=128)[:, 8:16, :],
                )
                nc.sync.dma_start(
                    out=wv_sb[:, :, :],
                    in_=wvT.rearrange("(k p) m -> p k m", p=128),
                )
                nc.sync.dma_start(out=tri_sb[:, :], in_=tri[:, :])
                nc.sync.dma_start(out=ident_sb[:, :], in_=ident[:, :])
                nc.vector.memset(ones_col[:, :], 1.0)

                def qk_pair(b, j, pair, xh, qh_sb, tabt):
                    pw = pstp.tile([128, 1024], F32, tag="pst", name=f"pw_{b}_{j}_{pair}")
                    psA = pw[:, 0:512]
                    psB = pw[:, 512:1024]
                    for mt, pst_ in ((pair, psA), (pair + 1, psB)):
                        for k in range(16):
                            nc.tensor.matmul(
                                pst_,
                                lhsT=(wqk_sb[:, k, 128 * mt : 128 * mt + 128]),
                                rhs=(xh[k // 8][:, k % 8, :]),
                                start=(k == 0),
                                stop=(k == 15),
                            )
                    ci = 0 if pair == 0 else 2
                    t1 = ropep.tile([128, 512], BF16, tag="t1", name=f"t1_{b}_{j}_{pair}")
                    t2 = ropep.tile([128, 512], BF16, tag="t2", name=f"t2_{b}_{j}_{pair}")
                    t3 = ropep.tile([128, 512], BF16, tag="t3", name=f"t3_{b}_{j}_{pair}")
                    t4 = ropep.tile([128, 512], BF16, tag="t4", name=f"t4_{b}_{j}_{pair}")
                    nc.vector.tensor_tensor(t1[:, :], psA, tabt[ci][:, :], MUL)
                    nc.vector.tensor_tensor(t2[:, :], psB, tabt[ci + 1][:, :], MUL)
                    nc.vector.tensor_tensor(t3[:, :], psB, tabt[ci][:, :], MUL)
                    nc.vector.tensor_tensor(t4[:, :], psA, tabt[ci + 1][:, :], MUL)
                    sl = stagep.tile([128, 512], BF16, tag="sl", name=f"sl_{b}_{j}_{pair}")
                    sh = stagep.tile([128, 512], BF16, tag="sh", name=f"sh_{b}_{j}_{pair}")
                    nc.vector.tensor_tensor(sl[:, :], t1[:, :], t2[:, :], SUB)
                    nc.vector.tensor_tensor(sh[:, :], t3[:, :], t4[:, :], ADD)
                    # repack: per-head [lo;hi] tiles for full-contract
                    # scores.  base tile index: q -> 0, k -> 2.
                    base = 0 if pair == 0 else 2
                    cs = slice(512 * j, 512 * (j + 1))
                    nc.sync.dma_start(out=qh_sb[0:64, base, cs], in_=sl[0:64, :])
                    nc.sync.dma_start(out=qh_sb[0:64, base + 1, cs], in_=sl[64:128, :])
                    nc.sync.dma_start(out=qh_sb[64:128, base, cs], in_=sh[0:64, :])
                    nc.sync.dma_start(out=qh_sb[64:128, base + 1, cs], in_=sh[64:128, :])

                def phase_a_chunk(b, j, qh_sb, vT_sb):
                    n0 = b * N + 512 * j
                    xh = xh_first if (b, j) == (0, 0) else load_x(b, j)
                    tabt = []
                    for ti in range(4):
                        tt = tabp.tile([128, 512], BF16, tag=f"tab{ti}", name=f"tab{ti}_{b}_{j}")
                        nc.sync.dma_start(out=tt[:, :], in_=tabs[ti, :, n0 : n0 + 512])
                        tabt.append(tt)
                    for pair in (0, 2):
                        qk_pair(b, j, pair, xh, qh_sb, tabt)
                    for mt in range(4):
                        pv = povp.tile([128, 256], F32, tag="pov", name=f"psV_{b}_{j}_{mt}")
                        for k in range(16):
                            nc.tensor.matmul(
                                pv[:, :],
                                lhsT=(xh[k // 8][:, k % 8, 128 * mt : 128 * mt + 128]),
                                rhs=(wv_sb[:, k, :]),
                                start=(k == 0),
                                stop=(k == 15),
                            )
                        nc.scalar.copy(vT_sb[:, 4 * j + mt, :], pv[:, :])

                def score_pair(b, h, j, m, qh_sb):
                    pw = pstp.tile(
                        [128, 1024], F32, tag="pst", name=f"stw_{b}_{h}_{j}_{m}"
                    )
                    for half in range(2):
                        t = 2 * m + half
                        sl = pw[:, 512 * half : 512 * half + 512]
                        diag = t // 4 == j
                        if diag:
                            # causal mask as a -30 additive bias matmul
                            # (identity stationary): no DVE pass after exp.
                            f0 = 128 * t - 512 * j
                            nc.tensor.matmul(
                                sl,
                                lhsT=ident_sb[:, :],
                                rhs=tri_sb[:, 512 - f0 : 1024 - f0],
                                start=True,
                                stop=False,
                            )
                        nc.tensor.matmul(
                            sl,
                            lhsT=(qh_sb[:, 2 + h, 128 * t : 128 * t + 128]),
                            rhs=(qh_sb[:, h, 512 * j : 512 * (j + 1)]),
                            start=not diag,
                            stop=True,
                        )
                    ptw = ptp.tile(
                        [128, 1024], BF16, tag="pt", name=f"pt_{b}_{h}_{j}_{m}"
                    )
                    nc.scalar.activation(ptw[:, :], pw[:, :], EXP)
                    return ptw

                def phase_b_unit(b, h, j, qh_sb, vT_sb):
                    ov = povp.tile([128, 512], F32, tag="pov", name=f"ov_{b}_{h}_{j}")
                    # row-sum path: j=0 accumulates per-tile on PE (PSUM),
                    # j>=1 accumulates elementwise on DVE -- balances PE
                    # against ACT/DVE in this phase.
                    on_pe = j == 0
                    rsum = None
                    rs_c = None
                    if on_pe:
                        rsum = psmallp.tile(
                            [1, 512], F32, tag="rsum", name=f"rsum_{b}_{h}_{j}"
                        )
                    else:
                        rs_c = rscp.tile(
                            [128, 512], F32, tag="rsc", name=f"rsc_{b}_{h}_{j}"
                        )
                    nt = 4 * j + 4
                    for m in range(nt // 2):
                        ptw = score_pair(b, h, j, m, qh_sb)
                        if on_pe:
                            for half in range(2):
                                t = 2 * m + half
                                nc.tensor.matmul(
                                    rsum[:, :],
                                    lhsT=ones_col[:, :],
                                    rhs=ptw[:, 512 * half : 512 * half + 512],
                                    start=(t == 0),
                                    stop=(t == nt - 1),
                                )
                        elif m == 0:
                            nc.vector.tensor_tensor(
                                rs_c[:, :], ptw[:, 0:512], ptw[:, 512:1024], ADD
                            )
                        else:
                            nc.vector.tensor_tensor(
                                rs_c[:, :], rs_c[:, :], ptw[:, 0:512], ADD
                            )
                            nc.vector.tensor_tensor(
                                rs_c[:, :], rs_c[:, :], ptw[:, 512:1024], ADD
                            )
                        for half in range(2):
                            t = 2 * m + half
                            nc.tensor.matmul(
                                ov[:, :],
                                lhsT=(vT_sb[:, t, 128 * h : 128 * h + 128]),
                                rhs=(ptw[:, 512 * half : 512 * half + 512]),
                                start=(t == 0),
                                stop=(t == nt - 1),
                            )
                    if not on_pe:
                        rs_bf = rscp.tile(
                            [128, 512], BF16, tag="rsbf", name=f"rsbf_{b}_{h}_{j}"
                        )
                        nc.scalar.copy(rs_bf[:, :], rs_c[:, :])
                        rsum = psmallp.tile(
                            [1, 512], F32, tag="rsum", name=f"rsum_{b}_{h}_{j}"
                        )
                        nc.tensor.matmul(
                            rsum[:, :],
                            lhsT=ones_col[:, :],
                            rhs=rs_bf[:, :],
                            start=True,
                            stop=True,
                        )
                    rinv = smallp.tile([1, 512], F32, tag="rinv", name=f"rinv_{b}_{h}_{j}")
                    nc.vector.reciprocal_approx_fast(rinv[:, :], rsum[:, :])
                    binv = smallp.tile(
                        [128, 512], F32, tag="binv", name=f"binv_{b}_{h}_{j}"
                    )
                    nc.gpsimd.partition_broadcast(binv[:, :], rinv[:, :])
                    ot = otsp.tile([128, 512], BF16, tag="ot", name=f"ot_{b}_{h}_{j}")
                    nc.vector.tensor_tensor(ot[:, :], ov[:, :], binv[:, :], MUL)
                    nc.sync.dma_start(
                        out=a2a_in[b][h][2 * j, :, :], in_=ot[:, 0:256]
                    )
                    nc.sync.dma_start(
                        out=a2a_in[b][h][2 * j + 1, :, :], in_=ot[:, 256:512]
                    )

                def emit_cc(b, h):
                    # per-(batch,head) reshard: all but the last collective
                    # hide under subsequent compute; the last hides under
                    # phase C's first chains.
                    nc.gpsimd.collective_compute(
                        "AllToAll",
                        mybir.AluOpType.bypass,
                        replica_groups=[list(range(NC))],
                        ins=[a2a_in[b][h].opt()],
                        outs=[a2a_out[b][h].opt()],
                    )

                # collectives are emitted one compute-unit late so the
                # gpsimd trigger doesn't block the next unit's
                # partition_broadcast.
                pending_cc = None
                for b in range(B):
                    qh_sb = persist.tile(
                        [128, 4, N], BF16, tag="qh", name=f"qh_b{b}"
                    )
                    vT_sb = persist.tile(
                        [128, 16, HPC * HD], BF16, tag="vT", name=f"vT_b{b}"
                    )
                    # ---------------- phase A: projection + RoPE ----------
                    for j in range(4):
                        phase_a_chunk(b, j, qh_sb, vT_sb)
                        if j == 0 and pending_cc is not None:
                            emit_cc(*pending_cc)
                            pending_cc = None

                    if b == 0:
                        # w_o cache fill: issued here so the 8MB transfer
                        # rides under b0's attention, clear of x loads.
                        nc.sync.dma_start(
                            out=wo_sb[:, :, :],
                            in_=woT.rearrange("(k p) d -> p k d", p=128),
                        )

                    # ---------------- phase B: attention ------------------
                    for h in range(HPC):
                        for j in range(4):
                            phase_b_unit(b, h, j, qh_sb, vT_sb)
                            if j == 0 and pending_cc is not None:
                                emit_cc(*pending_cc)
                                pending_cc = None
                        pending_cc = (b, h)
                if pending_cc is not None:
                    emit_cc(*pending_cc)
                    pending_cc = None

            # ---------------- phase C: o_proj ------------------------------
            # opin k-tile order is (h, src) -> woT rows are host-permuted to
            # match.  Output rows are staged in full-width tiles so each
            # 128-row block ships as one large DMA.
            with (
                tc.tile_pool(name="opin", bufs=1) as opinp,
                tc.tile_pool(name="outs", bufs=4) as outsp,
                tc.tile_pool(name="pc", bufs=4, space="PSUM") as pcp,
            ):
                opins = []
                for half in range(B):
                    opin = opinp.tile([128, 16, 256], BF16, tag=f"opin{half}", name=f"opin{half}")
                    for h in range(HPC):
                        nc.sync.dma_start(
                            out=opin[:, 8 * h : 8 * h + 8, :],
                            in_=a2a_out[half][h].rearrange("r p n -> p r n"),
                        )
                    opins.append(opin)
                def c_chain(half, ns, dc, pc, k0, k1):
                    opin = opins[half]
                    for k in range(k0, k1):
                        nc.tensor.matmul(
                            pc[:, :],
                            lhsT=(opin[:, k, 128 * ns : 128 * ns + 128]),
                            rhs=(wo_sb[:, k, 512 * dc : 512 * (dc + 1)]),
                            start=(k == 0),
                            stop=(k == 15),
                        )

                def c_finish(half, ns, dc, pc, ost):
                    nc.scalar.copy(ost[:, 512 * dc : 512 * (dc + 1)], pc[:, :])
                    r0 = 256 * half + 128 * ns
                    nc.sync.dma_start(
                        out=out[r0 : r0 + 128, 512 * dc : 512 * (dc + 1)],
                        in_=ost[:, 512 * dc : 512 * (dc + 1)],
                    )

                osts = {}
                for ns in range(2):
                    osts[0, ns] = outsp.tile([128, D], F32, tag="outs", name=f"os_0_{ns}")
                    for dc in range(4):
                        pc = pcp.tile([128, 512], F32, tag="pc", name=f"pc_0_{dc}_{ns}")
                        c_chain(0, ns, dc, pc, 0, 16)
                        c_finish(0, ns, dc, pc, osts[0, ns])
                # half 1: first chains split at the k=8 boundary -- the low
                # halves depend only on the earlier (b1,h0) collective, so
                # they fill the window while the last collective completes.
                pcs = {}
                for ns in range(2):
                    osts[1, ns] = outsp.tile([128, D], F32, tag="outs", name=f"os_1_{ns}")
                    for dc in range(2):
                        pcs[ns, dc] = pcp.tile([128, 512], F32, tag="pc", name=f"pc_1_{dc}_{ns}")
                        c_chain(1, ns, dc, pcs[ns, dc], 0, 8)
                for ns in range(2):
                    for dc in range(2):
                        c_chain(1, ns, dc, pcs[ns, dc], 8, 16)
                        c_finish(1, ns, dc, pcs[ns, dc], osts[1, ns])
                for ns in range(2):
                    for dc in range(2, 4):
                        pc = pcp.tile([128, 512], F32, tag="pc", name=f"pc_1_{dc}_{ns}")
                        c_chain(1, ns, dc, pc, 0, 16)
                        c_finish(1, ns, dc, pc, osts[1, ns])
    nc.compile()
    return nc


def _host_prep(x, w_qkv, w_o):
    bf = ml_dtypes.bfloat16
    xT = np.ascontiguousarray(x.reshape(BN, D).T).astype(bf)
    # o_proj k-tile order on device is (h_local, src_core): head g lives at
    # slot 8*(g % 2) + g // 2.
    woT_n = np.asarray(w_o).T.reshape(H, HD, D)
    perm = [2 * s + hl for hl in range(HPC) for s in range(NC)]
    woT = np.ascontiguousarray(woT_n[perm].reshape(INNER, D)).astype(bf)

    inv_freq = 1.0 / (ROPE_BASE ** (np.arange(0, HD, 2, dtype=np.float32) / HD))
    ang = np.arange(N, dtype=np.float32)[:, None] * inv_freq[None, :]
    cos_h = np.cos(ang).T.astype(np.float32)      # [64, N]
    sin_h = np.sin(ang).T.astype(np.float32)      # [64, N] (magnitude)
    # duplicated for the two heads packed per 128-row block
    cos2 = np.concatenate([cos_h, cos_h], axis=0)  # [128, N]
    sin2 = np.concatenate([sin_h, sin_h], axis=0)
    cos_f = np.tile(cos2, (1, B))
    sin_f = np.tile(sin2, (1, B))
    scale = np.float32(1.0 / np.sqrt(HD))
    tabs = np.ascontiguousarray(
        np.stack([cos_f * scale, sin_f * scale, cos_f, sin_f], axis=0)
    ).astype(bf)

    xf0 = np.ascontiguousarray(
        xT.reshape(16, 128, BN)[:, :, 0:512]
        .reshape(2, 8, 128, 512)
        .transpose(0, 2, 1, 3)
    )

    p = np.arange(128)[:, None]
    c = np.arange(1024)[None, :]
    # additive causal bias: 0 where visible, -30 where masked (exp -> ~0)
    tri = np.where(p <= c - 512, 0.0, -30.0).astype(bf)
    ident = np.eye(128, dtype=np.float32).astype(bf)

    in_maps = []
    for core in range(NC):
        h0 = core * HPC
        rq = slice(h0 * HD, (h0 + HPC) * HD)
        rk = slice(INNER + h0 * HD, INNER + (h0 + HPC) * HD)
        rv = slice(2 * INNER + h0 * HD, 2 * INNER + (h0 + HPC) * HD)
        wq = w_qkv[rq].reshape(HPC, HD, D)
        wk = w_qkv[rk].reshape(HPC, HD, D)
        # row order per block: [h0_lo, h1_lo | h0_hi, h1_hi] for q then k
        wqkT = np.ascontiguousarray(
            np.concatenate(
                [wq[0, :64], wq[1, :64], wq[0, 64:], wq[1, 64:],
                 wk[0, :64], wk[1, :64], wk[0, 64:], wk[1, 64:]], axis=0
            ).T
        ).astype(bf)
        wvT = np.ascontiguousarray(w_qkv[rv].T).astype(bf)
        in_maps.append(
            dict(xT=xT, wqkT=wqkT, wvT=wvT, woT=woT, tabs=tabs, tri=tri,
                 ident=ident, xf0=xf0)
        )
    return in_maps


def kernel(x, w_qkv, w_o, n_heads=None, head_dim=None, trace=False):
    global LAST_EXEC_NS, LAST_RESULTS
    x = np.asarray(x, dtype=np.float32)
    w_qkv = np.asarray(w_qkv, dtype=np.float32)
    w_o = np.asarray(w_o, dtype=np.float32)

    if "nc" not in _CACHE:
        _CACHE["nc"] = _build_program()
    nc = _CACHE["nc"]

    in_maps = _host_prep(x, w_qkv, w_o)
    res = None
    last_exc = None
    for attempt in range(4):
        try:
            res = bass_utils.run_bass_kernel_spmd(
                nc, in_maps, core_ids=list(range(NC)), trace=trace
            )
            break
        except Exception as e:  # transient compile_and_load / exec flakiness
            last_exc = e
            print(f"kernel attempt {attempt} failed: {e}", file=sys.stderr)
            time.sleep(5)
    if res is None:
        raise last_exc
    LAST_EXEC_NS = res.exec_time_ns
    LAST_RESULTS = res
    # core c returns [512, D]: rows 0:256 = batch0 rows 256c:256c+256,
    # rows 256:512 = batch1 rows 256c:256c+256.
    full = np.empty((B, N, D), dtype=np.float32)
    for c in range(NC):
        shard = res.results[c]["out"]
        full[0, 256 * c : 256 * c + 256] = shard[0:256]
        full[1, 256 * c : 256 * c + 256] = shard[256:512]
    return full


# revision 45
# speedup vs baseline: 1.0366x; 1.0197x over previous
"""Causal self-attention (B=2, N=2048, D=2048, H=16, hd=128) on 8 Trainium2
NeuronCores.

Strategy (tensor-parallel over heads, 2 heads/core), v2:
  - Host: transpose x / weights, build RoPE tables + triangular mask consts,
    slice w_qkv rows per head-group.
  - Device, per core (same SPMD program, different input data):
    Phase A: qkvT projection (bf16 matmuls, outputs in [d, n] layout) + RoPE
             (DVE mul/add on psum pairs) -> stage tiles -> SBUF->SBUF DMA
             repack into per-head [128=hd, N] q/k tiles (full-contract
             scores).
    Phase B: S.T = kh.T @ qh in ONE c=128 matmul per tile, P.T = exp(S.T)
             (ACT), causal mask via sliced triangular const (DVE), O.T
             accumulated as vT.T @ P.T (PE, PSUM accum).  Softmax denoms:
             ones-column matmul accumulated in PSUM over t (PE), fast
             reciprocal (DVE custom op), partition_broadcast (GPSIMD),
             final scale (DVE).
    Per-batch AllToAll (256-row chunks to each core) fired right after each
    batch's attention: b0's collective hides under b1's compute.
    Phase C: o_proj on the 2x256-row shard with w_o pre-cached in SBUF
             (8MB DMA issued during b0's attention); first half overlaps
             b1's collective.
  - Host: reassemble [b0 rows 256c:256c+256 | b1 rows 256c:256c+256].
"""

import sys
import time

import ml_dtypes
import numpy as np

sys.path.insert(0, "/opt/trn_rl_repo")

import concourse.bacc as bacc  # noqa: E402
import concourse.bass as bass  # noqa: E402
import concourse.mybir as mybir  # noqa: E402
import concourse.tile as tile  # noqa: E402
from concourse import bass_utils  # noqa: E402

F32 = mybir.dt.float32
BF16 = mybir.dt.bfloat16

B, N, D = 2, 2048, 2048
H, HD = 16, 128
NC = 8
HPC = H // NC          # heads per core
BN = B * N             # 4096
NSH = BN // NC         # output rows per core
INNER = H * HD
ROPE_BASE = 10000.0

_CACHE = {}

LAST_EXEC_NS = None
LAST_RESULTS = None


def _build_program():
    nc = bacc.Bacc(
        "TRN2",
        target_bir_lowering=False,
        debug=False,
        enable_asserts=False,
        num_devices=NC,
    )
    xT = nc.dram_tensor("xT", [D, BN], BF16, kind="ExternalInput").ap()
    wqkT = nc.dram_tensor("wqkT", [D, 4 * HD], BF16, kind="ExternalInput").ap()
    wvT = nc.dram_tensor("wvT", [D, HPC * HD], BF16, kind="ExternalInput").ap()
    woT = nc.dram_tensor("woT", [INNER, D], BF16, kind="ExternalInput").ap()
    tabs = nc.dram_tensor("tabs", [4, HD, BN], BF16, kind="ExternalInput").ap()
    tri = nc.dram_tensor("tri", [128, 1024], BF16, kind="ExternalInput").ap()
    ident = nc.dram_tensor("ident", [128, 128], BF16, kind="ExternalInput").ap()
    # contiguous copy of the first x chunk: the startup-critical DMA runs at
    # full rate instead of the strided gather pattern.
    xf0 = nc.dram_tensor("xf0", [2, 128, 8, 512], BF16, kind="ExternalInput").ap()
    out = nc.dram_tensor("out", [NSH, D], F32, kind="ExternalOutput").ap()
    a2a_in = [
        [
            nc.dram_tensor(f"a2a_in{b}_{h}", [NC, 128, 256], BF16).ap()
            for h in range(HPC)
        ]
        for b in range(B)
    ]
    a2a_out = [
        [
            nc.dram_tensor(f"a2a_out{b}_{h}", [NC, 128, 256], BF16).ap()
            for h in range(HPC)
        ]
        for b in range(B)
    ]

    MUL = mybir.AluOpType.mult
    ADD = mybir.AluOpType.add
    SUB = mybir.AluOpType.subtract
    EXP = mybir.ActivationFunctionType.Exp

    with tile.TileContext(nc, num_cores=NC) as tc:
        with (
            tc.tile_pool(name="const", bufs=1) as constp,
            tc.tile_pool(name="wqk", bufs=1) as wqkp,
            tc.tile_pool(name="wv", bufs=1) as wvp,
            tc.tile_pool(name="wo", bufs=1) as wop,
            tc.tile_pool(name="persist", bufs=1) as persist,
        ):
            wqk_sb = wqkp.tile([128, 16, 512], BF16, name="wqk_sb")
            wv_sb = wvp.tile([128, 16, 256], BF16, name="wv_sb")
            wo_sb = wop.tile([128, 16, D], BF16, name="wo_sb")
            tri_sb = constp.tile([128, 1024], BF16, name="tri_sb")
            ident_sb = constp.tile([128, 128], BF16, name="ident_sb")
            ones_col = constp.tile([128, 1], BF16, name="ones_col")

            with (
                tc.tile_pool(name="xt", bufs=5) as xtp,
                tc.tile_pool(name="tab", bufs=2) as tabp,
                tc.tile_pool(name="rope", bufs=2) as ropep,
                tc.tile_pool(name="stage", bufs=3) as stagep,
                tc.tile_pool(name="pt", bufs=5) as ptp,
                tc.tile_pool(name="small", bufs=2) as smallp,
                tc.tile_pool(name="ots", bufs=2) as otsp,
                tc.tile_pool(name="rsc", bufs=2) as rscp,
                tc.tile_pool(name="pst", bufs=2, space="PSUM") as pstp,
                tc.tile_pool(name="pov", bufs=2, space="PSUM") as povp,
                tc.tile_pool(name="psmall", bufs=2, space="PSUM") as psmallp,
            ):
                def load_x(b, j):
                    n0 = b * N + 512 * j
                    xh = []
                    for half in range(2):
                        t = xtp.tile(
                            [128, 8, 512], BF16, tag="xt", name=f"xt_{b}_{j}_{half}"
                        )
                        nc.sync.dma_start(
                            out=t[:, :, :],
                            in_=xT.rearrange("(k p) n -> p k n", p=128)[
                                :, 8 * half : 8 * half + 8, n0 : n0 + 512
                            ],
                        )
                        xh.append(t)
                    return xh

                # first x chunk before the weights: the first matmul chain
                # is gated on xh(0,0) + the low wqk half only.
                xh_first = []
                for half in range(2):
                    t = xtp.tile([128, 8, 512], BF16, tag="xt", name=f"xt_0_0_{half}")
                    nc.sync.dma_start(out=t[:, :, :], in_=xf0[half])
                    xh_first.append(t)
                nc.sync.dma_start(
                    out=wqk_sb[:, 0:8, :],
                    in_=wqkT.rearrange("(k p) m -> p k m", p=128)[:, 0:8, :],
                )
                nc.sync.dma_start(
                    out=wqk_sb[:, 8:16, :],
                    in_=wqkT.rearrange("(k p) m -> p k m", p=128)[:, 8:16, :],
                )
                nc.sync.dma_start(
                    out=wv_sb[:, :, :],
                    in_=wvT.rearrange("(k p) m -> p k m", p=128),
                )
                nc.sync.dma_start(out=tri_sb[:, :], in_=tri[:, :])
                nc.sync.dma_start(out=ident_sb[:, :], in_=ident[:, :])
                nc.vector.memset(ones_col[:, :], 1.0)

                def qk_pair(b, j, pair, xh, qh_sb, tabt):
                    pw = pstp.tile([128, 1024], F32, tag="pst", name=f"pw_{b}_{j}_{pair}")
                    psA = pw[:, 0:512]
                    psB = pw[:, 512:1024]
                    for mt, pst_ in ((pair, psA), (pair + 1, psB)):
                        for k in range(16):
                            nc.tensor.matmul(
                                pst_,
                                lhsT=(wqk_sb[:, k, 128 * mt : 128 * mt + 128]),
                                rhs=(xh[k // 8][:, k % 8, :]),
                                start=(k == 0),
                                stop=(k == 15),
                            )
                    ci = 0 if pair == 0 else 2
                    t1 = ropep.tile([128, 512], BF16, tag="t1", name=f"t1_{b}_{j}_{pair}")
                    t2 = ropep.tile([128, 512], BF16, tag="t2", name=f"t2_{b}_{j}_{pair}")
                    t3 = ropep.tile([128, 512], BF16, tag="t3", name=f"t3_{b}_{j}_{pair}")
                    t4 = ropep.tile([128, 512], BF16, tag="t4", name=f"t4_{b}_{j}_{pair}")
                    nc.vector.tensor_tensor(t1[:, :], psA, tabt[ci][:, :], MUL)
                    nc.vector.tensor_tensor(t2[:, :], psB, tabt[ci + 1][:, :], MUL)
                    nc.vector.tensor_tensor(t3[:, :], psB, tabt[ci][:, :], MUL)
                    nc.vector.tensor_tensor(t4[:, :], psA, tabt[ci + 1][:, :], MUL)
                    sl = stagep.tile([128, 512], BF16, tag="sl", name=f"sl_{b}_{j}_{pair}")
                    sh = stagep.tile([128, 512], BF16, tag="sh", name=f"sh_{b}_{j}_{pair}")
                    nc.vector.tensor_tensor(sl[:, :], t1[:, :], t2[:, :], SUB)
                    nc.vector.tensor_tensor(sh[:, :], t3[:, :], t4[:, :], ADD)
                    # repack: per-head [lo;hi] tiles for full-contract
                    # scores.  base tile index: q -> 0, k -> 2.
                    base = 0 if pair == 0 else 2
                    cs = slice(512 * j, 512 * (j + 1))
                    nc.sync.dma_start(out=qh_sb[0:64, base, cs], in_=sl[0:64, :])
                    nc.sync.dma_start(out=qh_sb[0:64, base + 1, cs], in_=sl[64:128, :])
                    nc.sync.dma_start(out=qh_sb[64:128, base, cs], in_=sh[0:64, :])
                    nc.sync.dma_start(out=qh_sb[64:128, base + 1, cs], in_=sh[64:128, :])

                def phase_a_chunk(b, j, qh_sb, vT_sb):
                    n0 = b * N + 512 * j
                    xh = xh_first if (b, j) == (0, 0) else load_x(b, j)
                    tabt = []
                    for ti in range(4):
                        tt = tabp.tile([128, 512], BF16, tag=f"tab{ti}", name=f"tab{ti}_{b}_{j}")
                        nc.sync.dma_start(out=tt[:, :], in_=tabs[ti, :, n0 : n0 + 512])
                        tabt.append(tt)
                    for pair in (0, 2):
                        qk_pair(b, j, pair, xh, qh_sb, tabt)
                    for mt in range(4):
                        pv = povp.tile([128, 256], F32, tag="pov", name=f"psV_{b}_{j}_{mt}")
                        for k in range(16):
                            nc.tensor.matmul(
                                pv[:, :],
                                lhsT=(xh[k // 8][:, k % 8, 128 * mt : 128 * mt + 128]),
                                rhs=(wv_sb[:, k, :]),
                                start=(k == 0),
                                stop=(k == 15),
                            )
                        nc.scalar.copy(vT_sb[:, 4 * j + mt, :], pv[:, :])

                def score_pair(b, h, j, m, qh_sb):
                    pw = pstp.tile(
                        [128, 1024], F32, tag="pst", name=f"stw_{b}_{h}_{j}_{m}"
                    )
                    for half in range(2):
                        t = 2 * m + half
                        sl = pw[:, 512 * half : 512 * half + 512]
                        diag = t // 4 == j
                        if diag:
                            # causal mask as a -30 additive bias matmul
                            # (identity stationary): no DVE pass after exp.
                            f0 = 128 * t - 512 * j
                            nc.tensor.matmul(
                                sl,
                                lhsT=ident_sb[:, :],
                                rhs=tri_sb[:, 512 - f0 : 1024 - f0],
                                start=True,
                                stop=False,
                            )
                        nc.tensor.matmul(
                            sl,
                            lhsT=(qh_sb[:, 2 + h, 128 * t : 128 * t + 128]),
                            rhs=(qh_sb[:, h, 512 * j : 512 * (j + 1)]),
                            start=not diag,
                            stop=True,
                        )
                    ptw = ptp.tile(
                        [128, 1024], BF16, tag="pt", name=f"pt_{b}_{h}_{j}_{m}"
                    )
                    nc.scalar.activation(ptw[:, :], pw[:, :], EXP)
                    return ptw

                def phase_b_unit(b, h, j, qh_sb, vT_sb):
                    ov = povp.tile([128, 512], F32, tag="pov", name=f"ov_{b}_{h}_{j}")
                    # row-sum path: j=0 accumulates per-tile on PE (PSUM),
                    # j>=1 accumulates elementwise on DVE -- balances PE
                    # against ACT/DVE in this phase.
                    on_pe = j == 0
                    rsum = None
                    rs_c = None
                    if on_pe:
                        rsum = psmallp.tile(
                            [1, 512], F32, tag="rsum", name=f"rsum_{b}_{h}_{j}"
                        )
                    else:
                        rs_c = rscp.tile(
                            [128, 512], F32, tag="rsc", name=f"rsc_{b}_{h}_{j}"
                        )
                    nt = 4 * j + 4
                    for m in range(nt // 2):
                        ptw = score_pair(b, h, j, m, qh_sb)
                        if on_pe:
                            for half in range(2):
                                t = 2 * m + half
                                nc.tensor.matmul(
                                    rsum[:, :],
                                    lhsT=ones_col[:, :],
                                    rhs=ptw[:, 512 * half : 512 * half + 512],
                                    start=(t == 0),
                                    stop=(t == nt - 1),
                                )
                        elif m == 0:
                            nc.vector.tensor_tensor(
                                rs_c[:, :], ptw[:, 0:512], ptw[:, 512:1024], ADD
                            )
                        else:
                            nc.vector.tensor_tensor(
                                rs_c[:, :], rs_c[:, :], ptw[:, 0:512], ADD
                            )
                            nc.vector.tensor_tensor(
                                rs_c[:, :], rs_c[:, :], ptw[:, 512:1024], ADD
                            )
                        for half in range(2):
                            t = 2 * m + half
                            nc.tensor.matmul(
                                ov[:, :],
                                lhsT=(vT_sb[:, t, 128 * h : 128 * h + 128]),
                                rhs=(ptw[:, 512 * half : 512 * half + 512]),
                                start=(t == 0),
                                stop=(t == nt - 1),
                            )
                    if not on_pe:
                        rs_bf = rscp.tile(
                            [128, 512], BF16, tag="rsbf", name=f"rsbf_{b}_{h}_{j}"
                        )
                        nc.scalar.copy(rs_bf[:, :], rs_c[:, :])
                        rsum = psmallp.tile(
                            [1, 512], F32, tag="rsum", name=f"rsum_{b}_{h}_{j}"
                        )
                        nc.tensor.matmul(
                            rsum[:, :],
                            lhsT=ones_col[:, :],
                            rhs=rs_bf[:, :],
                            start=True,
                            stop=True,
                        )
                    rinv = smallp.tile([1, 512], F32, tag="rinv", name=f"rinv_{b}_{h}_{j}")
                    nc.vector.reciprocal_approx_fast(rinv[:, :], rsum[:, :])
                    binv = smallp.tile(
                        [128, 512], F32, tag="binv", name=f"binv_{b}_{h}_{j}"
                    )
                    nc.gpsimd.partition_broadcast(binv[:, :], rinv[:, :])
                    ot = otsp.tile([128, 512], BF16, tag="ot", name=f"ot_{b}_{h}_{j}")
                    nc.vector.tensor_tensor(ot[:, :], ov[:, :], binv[:, :], MUL)
                    nc.sync.dma_start(
                        out=a2a_in[b][h][2 * j, :, :], in_=ot[:, 0:256]
                    )
                    nc.sync.dma_start(
                        out=a2a_in[b][h][2 * j + 1, :, :], in_=ot[:, 256:512]
                    )

                def emit_cc(b, h):
                    # per-(batch,head) reshard: all but the last collective
                    # hide under subsequent compute; the last hides under
                    # phase C's first chains.
                    nc.gpsimd.collective_compute(
                        "AllToAll",
                        mybir.AluOpType.bypass,
                        replica_groups=[list(range(NC))],
                        ins=[a2a_in[b][h].opt()],
                        outs=[a2a_out[b][h].opt()],
                    )

                # collectives are emitted one compute-unit late so the
                # gpsimd trigger doesn't block the next unit's
                # partition_broadcast.
                pending_cc = None
                for b in range(B):
                    qh_sb = persist.tile(
                        [128, 4, N], BF16, tag="qh", name=f"qh_b{b}"
                    )
                    vT_sb = persist.tile(
                        [128, 16, HPC * HD], BF16, tag="vT", name=f"vT_b{b}"
                    )
                    # ---------------- phase A: projection + RoPE ----------
                    for j in range(4):
                        phase_a_chunk(b, j, qh_sb, vT_sb)
                        if j == 0 and pending_cc is not None:
                            emit_cc(*pending_cc)
                            pending_cc = None

                    if b == 0:
                        # w_o cache fill: issued here so the 8MB transfer
                        # rides under b0's attention, clear of x loads.
                        nc.sync.dma_start(
                            out=wo_sb[:, :, :],
                            in_=woT.rearrange("(k p) d -> p k d", p=128),
                        )

                    # ---------------- phase B: attention ------------------
                    for h in range(HPC):
                        for j in range(4):
                            phase_b_unit(b, h, j, qh_sb, vT_sb)
                            if j == 0 and pending_cc is not None:
                                emit_cc(*pending_cc)
                                pending_cc = None
                        pending_cc = (b, h)
                if pending_cc is not None:
                    emit_cc(*pending_cc)
                    pending_cc = None

            # ---------------- phase C: o_proj ------------------------------
            # opin k-tile order is (h, src) -> woT rows are host-permuted to
            # match.  Output rows are staged in full-width tiles so each
            # 128-row block ships as one large DMA.
            with (
                tc.tile_pool(name="opin", bufs=1) as opinp,
                tc.tile_pool(name="outs", bufs=4) as outsp,
                tc.tile_pool(name="pc", bufs=4, space="PSUM") as pcp,
            ):
                opins = []
                for half in range(B):
                    opin = opinp.tile([128, 16, 256], BF16, tag=f"opin{half}", name=f"opin{half}")
                    for h in range(HPC):
                        nc.sync.dma_start(
                            out=opin[:, 8 * h : 8 * h + 8, :],
                            in_=a2a_out[half][h].rearrange("r p n -> p r n"),
                        )
                    opins.append(opin)
                def c_chain(half, ns, dc, pc, k0, k1):
                    opin = opins[half]
                    for k in range(k0, k1):
                        nc.tensor.matmul(
                            pc[:, :],
                            lhsT=(opin[:, k, 128 * ns : 128 * ns + 128]),
                            rhs=(wo_sb[:, k, 512 * dc : 512 * (dc + 1)]),
                            start=(k == 0),
                            stop=(k == 15),
                        )

                def c_finish(half, ns, dc, pc, ost):
                    nc.scalar.copy(ost[:, 512 * dc : 512 * (dc + 1)], pc[:, :])
                    r0 = 256 * half + 128 * ns
                    nc.sync.dma_start(
                        out=out[r0 : r0 + 128, 512 * dc : 512 * (dc + 1)],
                        in_=ost[:, 512 * dc : 512 * (dc + 1)],
                    )

                osts = {}
                for ns in range(2):
                    osts[0, ns] = outsp.tile([128, D], F32, tag="outs", name=f"os_0_{ns}")
                    for dc in range(4):
                        pc = pcp.tile([128, 512], F32, tag="pc", name=f"pc_0_{dc}_{ns}")
                        c_chain(0, ns, dc, pc, 0, 16)
                        c_finish(0, ns, dc, pc, osts[0, ns])
                # half 1: first chains split at the k=8 boundary -- the low
                # halves depend only on the earlier (b1,h0) collective, so
                # they fill the window while the last collective completes.
                pcs = {}
                for ns in range(2):
                    osts[1, ns] = outsp.tile([128, D], F32, tag="outs", name=f"os_1_{ns}")
                    for dc in range(2):
                        pcs[ns, dc] = pcp.tile([128, 512], F32, tag="pc", name=f"pc_1_{dc}_{ns}")
                        c_chain(1, ns, dc, pcs[ns, dc], 0, 8)
                for ns in range(2):
                    for dc in range(2):
                        c_chain(1, ns, dc, pcs[ns, dc], 8, 16)
                        c_finish(1, ns, dc, pcs[ns, dc], osts[1, ns])
                for ns in range(2):
                    for dc in range(2, 4):
                        pc = pcp.tile([128, 512], F32, tag="pc", name=f"pc_1_{dc}_{ns}")
                        c_chain(1, ns, dc, pc, 0, 16)
                        c_finish(1, ns, dc, pc, osts[1, ns])
    nc.compile()
    return nc


def _host_prep(x, w_qkv, w_o):
    bf = ml_dtypes.bfloat16
    xT = np.ascontiguousarray(x.reshape(BN, D).T).astype(bf)
    # o_proj k-tile order on device is (h_local, src_core): head g lives at
    # slot 8*(g % 2) + g // 2.
    woT_n = np.asarray(w_o).T.reshape(H, HD, D)
    perm = [2 * s + hl for hl in range(HPC) for s in range(NC)]
    woT = np.ascontiguousarray(woT_n[perm].reshape(INNER, D)).astype(bf)

    inv_freq = 1.0 / (ROPE_BASE ** (np.arange(0, HD, 2, dtype=np.float32) / HD))
    ang = np.arange(N, dtype=np.float32)[:, None] * inv_freq[None, :]
    cos_h = np.cos(ang).T.astype(np.float32)      # [64, N]
    sin_h = np.sin(ang).T.astype(np.float32)      # [64, N] (magnitude)
    # duplicated for the two heads packed per 128-row block
    cos2 = np.concatenate([cos_h, cos_h], axis=0)  # [128, N]
    sin2 = np.concatenate([sin_h, sin_h], axis=0)
    cos_f = np.tile(cos2, (1, B))
    sin_f = np.tile(sin2, (1, B))
    scale = np.float32(1.0 / np.sqrt(HD))
    tabs = np.ascontiguousarray(
        np.stack([cos_f * scale, sin_f * scale, cos_f, sin_f], axis=0)
    ).astype(bf)

    xf0 = np.ascontiguousarray(
        xT.reshape(16, 128, BN)[:, :, 0:512]
        .reshape(2, 8, 128, 512)
        .transpose(0, 2, 1, 3)
    )

    p = np.arange(128)[:, None]
    c = np.arange(1024)[None, :]
    # additive causal bias: 0 where visible, -30 where masked (exp -> ~0)
    tri = np.where(p <= c - 512, 0.0, -30.0).astype(bf)
    ident = np.eye(128, dtype=np.float32).astype(bf)

    in_maps = []
    for core in range(NC):
        h0 = core * HPC
        rq = slice(h0 * HD, (h0 + HPC) * HD)
        rk = slice(INNER + h0 * HD, INNER + (h0 + HPC) * HD)
        rv = slice(2 * INNER + h0 * HD, 2 * INNER + (h0 + HPC) * HD)
        wq = w_qkv[rq].reshape(HPC, HD, D)
        wk = w_qkv[rk].reshape(HPC, HD, D)
        # row order per block: [h0_lo, h1_lo | h0_hi, h1_hi] for q then k
        wqkT = np.ascontiguousarray(
            np.concatenate(
                [wq[0, :64], wq[1, :64], wq[0, 64:], wq[1, 64:],
                 wk[0, :64], wk[1, :64], wk[0, 64:], wk[1, 64:]], axis=0
            ).T
        ).astype(bf)
        wvT = np.ascontiguousarray(w_qkv[rv].T).astype(bf)
        in_maps.append(
            dict(xT=xT, wqkT=wqkT, wvT=wvT, woT=woT, tabs=tabs, tri=tri,
                 ident=ident, xf0=xf0)
        )
    return in_maps


def kernel(x, w_qkv, w_o, n_heads=None, head_dim=None, trace=False):
    global LAST_EXEC_NS, LAST_RESULTS
    x = np.asarray(x, dtype=np.float32)
    w_qkv = np.asarray(w_qkv, dtype=np.float32)
    w_o = np.asarray(w_o, dtype=np.float32)

    if "nc" not in _CACHE:
        _CACHE["nc"] = _build_program()
    nc = _CACHE["nc"]

    in_maps = _host_prep(x, w_qkv, w_o)
    res = None
    last_exc = None
    for attempt in range(4):
        try:
            res = bass_utils.run_bass_kernel_spmd(
                nc, in_maps, core_ids=list(range(NC)), trace=trace
            )
            break
        except Exception as e:  # transient compile_and_load / exec flakiness
            last_exc = e
            print(f"kernel attempt {attempt} failed: {e}", file=sys.stderr)
            time.sleep(5)
    if res is None:
        raise last_exc
    LAST_EXEC_NS = res.exec_time_ns
    LAST_RESULTS = res
    # core c returns [512, D]: rows 0:256 = batch0 rows 256c:256c+256,
    # rows 256:512 = batch1 rows 256c:256c+256.
    full = np.empty((B, N, D), dtype=np.float32)
    for c in range(NC):
        shard = res.results[c]["out"]
        full[0, 256 * c : 256 * c + 256] = shard[0:256]
        full[1, 256 * c : 256 * c + 256] = shard[256:512]
    return full
